# revision 26
# baseline (speedup 1.0000x reference)
"""GCN layer (nn_GCNReg) on 8 Trainium2 NeuronCores.

Strategy (graph/data parallel, per sharding hint):
  - Nodes are partitioned across 8 cores by destination range (49 tiles of
    128 nodes per core). Edges are routed to the core owning their dst and
    sorted by dst on the host, which also emits the dinv-scaled f16 gather
    table xs = deg^{-1/2} * x (the same host routing pass already computes
    degrees from edge_index).
  - Math: out = relu(dinv_dst * ((sum_e xs[src_e]) @ W1^T) + b1) @ W2^T + b2.
    The W1 transform commutes with the aggregation, so each core only
    transforms its own 6272 aggregated rows.
  - Self-loop messages are contiguous rows, so they bypass the gather: each
    core keeps its own xs slice resident in SBUF and feeds it straight into
    the scatter matmuls with an identity dstloc column.
  - Remaining per-edge rows are fetched with gpsimd.dma_gather spread over 4
    SWDGE queues (each queue's ring drains on its own SDMA engine; the drain
    is descriptor-count-bound at ~8.4 ns/row/queue). int16 indices only
    reach 32768 rows, so edges are split into src-parity sides (idx = src>>1,
    stride 2 rows via elem_step). One-hot scatter matrices are built with
    batched is_equal tensor_tensor ops; agg^T = M^T S accumulates in PSUM.
"""

import sys

import numpy as np

for _p in ("/opt/trn_rl_repo", "/opt/pypackages"):
    if _p not in sys.path:
        sys.path.append(_p)

import concourse.bass as bass
import concourse.tile as tile
from concourse import bacc, mybir
from concourse.bass_utils import run_bass_kernel_spmd

N = 50000
D = 128
HID = 128
ODIM = 8
CORES = 8
TILE = 128
TPC = 49                      # tiles per core (core 7: 48 real + 1 dummy)
NPC = TPC * TILE              # 6272 nodes per core
NP = CORES * NPC              # 50176 padded node count
BATCH_BWS = [512] * 12 + [128]   # 12*512 + 128 = 6272
NBATCH = len(BATCH_BWS)
PAD_DL = 999.0                # dstlocal sentinel: matches no iota column
GCH = 8                       # chunks per dma_gather call (1024 idx ring cap)
NQUEUES = 4                   # SWDGE queues (ucode max)
SGROUP = 8                    # S matrices built per tensor_tensor op

F16 = np.float16
F32 = np.float32


def _preprocess(x, edge_index):
    """Route/sort edges; build gather table + per-core index/dstloc arrays."""
    src = np.asarray(edge_index[0], dtype=np.int64)
    dst = np.asarray(edge_index[1], dtype=np.int64)
    order = np.argsort(dst, kind="stable")
    ssrc = src[order].astype(np.int32)
    sdst = dst[order].astype(np.int32)

    # degree includes the self-loop for every real node
    counts = np.bincount(dst, minlength=NP) + 1
    counts[N:] = 1                           # pad nodes -> deg 1, no edges
    dinv = 1.0 / np.sqrt(counts.astype(F32))  # [NP]
    ecounts = np.bincount(sdst, minlength=NP)
    ptr = np.zeros(NP + 1, dtype=np.int64)
    ptr[1:] = np.cumsum(ecounts)

    xs = np.zeros((NP, D), dtype=F16)
    xs[:N] = (np.asarray(x, dtype=F32) * dinv[:N, None]).astype(F16)

    # per (core, batch, side): half-idx list + batch-relative dst list
    per = [[None] * (2 * NBATCH) for _ in range(CORES)]
    for c in range(CORES):
        for b in range(NBATCH):
            base = c * NPC + b * 512
            bw = BATCH_BWS[b]
            lo_e = ptr[base]
            hi_e = ptr[min(base + bw, NP)]
            s = ssrc[lo_e:hi_e]
            dl = (sdst[lo_e:hi_e] - base).astype(np.int32)
            m = (s & 1) == 0
            per[c][2 * b] = (s[m] >> 1, dl[m])
            per[c][2 * b + 1] = (s[~m] >> 1, dl[~m])

    # uniform chunk counts + mm-entry lists; buffer layout per batch:
    # [nself identity chunks][side0 k chunks][side1 k chunks]
    meta = {"batches": []}
    nidx16_tot = 0
    nmm_tot = 0         # matmul entries == dstloc columns
    for b in range(NBATCH):
        nself = (BATCH_BWS[b] + TILE - 1) // TILE
        ent = {"bw": BATCH_BWS[b], "nself": nself, "sides": []}
        for side in range(2):
            cmax = max(len(per[c][2 * b + side][0]) for c in range(CORES))
            cmax = max(cmax, 1)
            k = (cmax + TILE - 1) // TILE
            ent["sides"].append({"k": k, "idx_off16": nidx16_tot})
            nidx16_tot += (k * TILE) // 16

        mm = []  # (buf_ci, tile_j, dstloc_col, is_self)
        for ci in range(nself):     # identity chunks first
            mm.append((ci, ci, nmm_tot, True))
            nmm_tot += 1
        klo = ent["sides"][0]["k"]
        for side in range(2):
            sd = ent["sides"][side]
            k = sd["k"]
            lo_span = np.full(k, np.inf)
            hi_span = np.full(k, -np.inf)
            for c in range(CORES):
                _, dl_l = per[c][2 * b + side]
                n = len(dl_l)
                if n == 0:
                    continue
                nk = (n + TILE - 1) // TILE
                starts = np.arange(nk) * TILE
                mn = np.minimum.reduceat(dl_l, starts)
                mx = np.maximum.reduceat(dl_l, starts)
                lo_span[:nk] = np.minimum(lo_span[:nk], mn)
                hi_span[:nk] = np.maximum(hi_span[:nk], mx)
            for ci in range(k):
                if not np.isfinite(lo_span[ci]):
                    continue   # all-pad chunk on every core: no matmul
                j0 = int(lo_span[ci]) // TILE
                j1 = int(hi_span[ci]) // TILE
                buf_ci = nself + (ci if side == 0 else klo + ci)
                for j in range(j0, j1 + 1):
                    mm.append((buf_ci, j, nmm_tot, False))
                    nmm_tot += 1
        ent["mm"] = mm
        meta["batches"].append(ent)
    meta["nidx16"] = nidx16_tot
    meta["nmm"] = nmm_tot
    meta["maxch"] = max(
        e["nself"] + e["sides"][0]["k"] + e["sides"][1]["k"]
        for e in meta["batches"]
    )

    # per-core packed arrays
    dstloc = np.full((CORES, 128, nmm_tot), PAD_DL, dtype=F16)
    srcidx = np.zeros((CORES, 128, nidx16_tot), dtype=np.int16)
    dinvB = np.empty((CORES, 128, NPC), dtype=F16)
    iota_col = np.arange(TILE, dtype=F32)
    for c in range(CORES):
        dinvB[c] = np.broadcast_to(dinv[c * NPC : (c + 1) * NPC], (128, NPC))
        for b in range(NBATCH):
            ent = meta["batches"][b]
            nself = ent["nself"]
            dls = [np.broadcast_to(iota_col, (nself, TILE))]
            for side in range(2):
                sd = ent["sides"][side]
                idx_l, dl_l = per[c][2 * b + side]
                n = len(idx_l)
                k = sd["k"]
                tot = k * TILE
                idx = np.zeros(tot, dtype=np.int16)
                idx[:n] = idx_l.astype(np.int16)
                srcidx[c][:, sd["idx_off16"] : sd["idx_off16"] + tot // 16] = (
                    np.tile(idx.reshape(tot // 16, 16).T, (8, 1))
                )
                dl = np.full(tot, PAD_DL, dtype=F32)
                dl[:n] = dl_l.astype(F32)
                dls.append(dl.reshape(k, TILE))
            dl_all = np.concatenate(dls, axis=0)  # [nself+klo+khi, 128]
            for buf_ci, j, col, is_self in ent["mm"]:
                if is_self:
                    dstloc[c][:, col] = iota_col.astype(F16)
                else:
                    dstloc[c][:, col] = (dl_all[buf_ci] - j * TILE).astype(F16)
    # entries outside [0,128) (other tile's edges / pads) match no iota col
    dstloc[np.logical_or(dstloc < 0, dstloc >= TILE)] = PAD_DL

    # per-core resident xs slice for the self-loop messages
    xself = np.ascontiguousarray(
        xs.reshape(CORES, TPC, 128, D).transpose(0, 2, 1, 3)
    )  # [CORES, 128, TPC, D]: node c*NPC + a*128 + p -> [c][p, a, :]

    return meta, xs, xself, dstloc, srcidx, dinvB


def _bc_mid(ap2d, g):
    """[128, W] AP -> [128, g, W] with a step-0 middle dim."""
    return bass.AP(ap2d.tensor, ap2d.offset, [ap2d.ap[0], [0, g], ap2d.ap[1]])


def _build_program(meta):
    nc = bacc.Bacc("TRN2", target_bir_lowering=False, debug=False,
                   num_devices=CORES, num_swdge_queues=NQUEUES)
    dt = mybir.dt

    xs_d = nc.dram_tensor("xs", [NP, D], dt.float16, kind="ExternalInput")
    xself_d = nc.dram_tensor("xself", [128, TPC, D], dt.float16,
                             kind="ExternalInput")
    dstloc_d = nc.dram_tensor("dstloc", [128, meta["nmm"]], dt.float16,
                              kind="ExternalInput")
    srcidx_d = nc.dram_tensor("srcidx", [128, meta["nidx16"]], dt.int16,
                              kind="ExternalInput")
    dinvB_d = nc.dram_tensor("dinvB", [128, NPC], dt.float16,
                             kind="ExternalInput")
    iota_d = nc.dram_tensor("iota", [128, 128], dt.float16, kind="ExternalInput")
    w1t_d = nc.dram_tensor("w1t", [D, HID], dt.float32, kind="ExternalInput")
    b1_d = nc.dram_tensor("b1c", [HID, 1], dt.float32, kind="ExternalInput")
    w2t_d = nc.dram_tensor("w2t", [HID, ODIM], dt.float16, kind="ExternalInput")
    b2_d = nc.dram_tensor("b2c", [ODIM, 1], dt.float32, kind="ExternalInput")
    out_d = nc.dram_tensor("out", [ODIM, NPC], dt.float32, kind="ExternalOutput")

    with tile.TileContext(nc) as tc:
        with (
            tc.tile_pool(name="const", bufs=1) as cpool,
            tc.tile_pool(name="msg", bufs=3) as msg_pool,
            tc.tile_pool(name="smat", bufs=4) as s_pool,
            tc.tile_pool(name="eptmp", bufs=2) as ep_pool,
            tc.tile_pool(name="psA", bufs=3, space="PSUM") as psA,
            tc.tile_pool(name="psZ", bufs=2, space="PSUM") as psZ,
            tc.tile_pool(name="psO", bufs=2, space="PSUM") as psO,
        ):
            # dummy Pool op up front so the Q7 ucode library loads during DMAs
            zeros_t = cpool.tile([1, 512], dt.float16, tag="zeros")
            nc.gpsimd.memset(zeros_t[:], 0.0)

            # idx table as two separate tiles: the early gathers depend only
            # on the small first tile, not the whole-table transfer (~10us)
            cut = meta["batches"][2]["sides"][0]["idx_off16"]
            idxA_t = cpool.tile([128, cut], dt.int16, tag="srcidxA")
            # SWDGE path: its completion sem is disjoint from the HWDGE
            # lane ticks, so the first gather's wait set stays minimal
            nc.gpsimd.dma_start(idxA_t[:], srcidx_d.ap()[:, :cut])
            idxB_t = cpool.tile([128, meta["nidx16"] - cut], dt.int16,
                                tag="srcidxB")
            nc.sync.dma_start(idxB_t[:], srcidx_d.ap()[:, cut:])

            iota_t = cpool.tile([128, 128], dt.float16, tag="iota")
            nc.sync.dma_start(iota_t[:], iota_d.ap())
            dstloc_t = cpool.tile([128, meta["nmm"]], dt.float16, tag="dstloc")
            nc.sync.dma_start(dstloc_t[:], dstloc_d.ap())
            # epilogue-only tensors ride the scalar engine's HWDGE queue
            xself_t = cpool.tile([128, TPC, D], dt.float16, tag="xself")
            w1t_t = cpool.tile([D, HID], dt.float32, tag="w1t")
            nc.scalar.dma_start(w1t_t[:], w1t_d.ap())
            b1_t = cpool.tile([HID, 1], dt.float32, tag="b1")
            nc.scalar.dma_start(b1_t[:], b1_d.ap())
            w2t_t = cpool.tile([HID, ODIM], dt.float16, tag="w2t")
            nc.scalar.dma_start(w2t_t[:], w2t_d.ap())
            b2_t = cpool.tile([ODIM, 1], dt.float32, tag="b2")
            nc.scalar.dma_start(b2_t[:], b2_d.ap())
            dinvB_t = cpool.tile([128, NPC], dt.float16, tag="dinvB")

            # ---- gathers + one-hot scatter matmuls + epilogue ----
            out_acc = cpool.tile([ODIM, NPC], dt.float32, tag="outacc")
            pend = []

            def _epilogue(b, bw, agg_ps):
                agg_sb = ep_pool.tile([128, 512], dt.float32, tag="aggsb")
                nc.scalar.copy(agg_sb[:, :bw], agg_ps[:, :bw])
                z_ps = psZ.tile([128, 512], dt.float32, tag="z")
                nc.tensor.matmul(out=z_ps[:, :bw], lhsT=w1t_t[:],
                                 rhs=agg_sb[:, :bw], start=True, stop=True)
                z2_sb = ep_pool.tile([128, 512], dt.float32, tag="z2")
                nc.vector.tensor_tensor(
                    out=z2_sb[:, :bw],
                    in0=z_ps[:, :bw],
                    in1=dinvB_t[:, b * 512 : b * 512 + bw],
                    op=mybir.AluOpType.mult,
                )
                h_sb = ep_pool.tile([128, 512], dt.float16, tag="h")
                nc.scalar.activation(h_sb[:, :bw], z2_sb[:, :bw],
                                     mybir.ActivationFunctionType.Relu,
                                     bias=b1_t[:])
                o_ps = psO.tile([ODIM, 512], dt.float32, tag="o")
                nc.tensor.matmul(out=o_ps[:, :bw], lhsT=w2t_t[:],
                                 rhs=h_sb[:, :bw], start=True, stop=True)
                nc.vector.tensor_scalar(
                    out=out_acc[:, b * 512 : b * 512 + bw],
                    in0=o_ps[:, :bw],
                    scalar1=b2_t[:],
                    scalar2=None,
                    op0=mybir.AluOpType.add,
                )
                nc.sync.dma_start(
                    out_d.ap()[:, b * 512 : b * 512 + bw],
                    out_acc[:, b * 512 : b * 512 + bw],
                )
            # src-parity side views: row stride 2 (512B), side s offset s*D
            xs_ap = xs_d.ap()
            side_aps = [
                bass.AP(xs_ap.tensor, s * D, [[2 * D, NP // 2], [1, D]])
                for s in range(2)
            ]
            gq = 0  # gather queue round-robin

            for b in range(NBATCH):
                ent = meta["batches"][b]
                bw = ent["bw"]
                nself = ent["nself"]
                klo = ent["sides"][0]["k"]
                buf = msg_pool.tile([128, meta["maxch"], D], dt.float16,
                                    tag="msg")
                for side, c0 in ((0, nself), (1, nself + klo)):
                    sd = ent["sides"][side]
                    k = sd["k"]
                    for p0 in range(0, k, GCH):
                        pk = min(GCH, k - p0)
                        off = sd["idx_off16"] + (p0 * TILE) // 16
                        it = idxA_t if off < cut else None
                        if it is None:
                            it, off = idxB_t, off - cut
                        nc.gpsimd.dma_gather(
                            out_ap=buf[:, c0 + p0 : c0 + p0 + pk, :],
                            in_ap=side_aps[side],
                            idxs_ap=it[:, off : off + (pk * TILE) // 16],
                            num_idxs=pk * TILE,
                            num_idxs_reg=pk * TILE,
                            elem_size=D,
                            elem_step=2 * D,
                            single_packet=True,
                            queue_num=gq % NQUEUES,
                        )
                        gq += 1

                if b == 0:
                    # big epilogue loads go out only after the first gathers'
                    # idx slice has cleared the shared SDMA engines
                    nc.scalar.dma_start(xself_t[:], xself_d.ap())
                    nc.scalar.dma_start(dinvB_t[:], dinvB_d.ap())

                agg_ps = psA.tile([128, 512], dt.float32, tag="agg")
                nc.tensor.matmul(
                    out=agg_ps[:], lhsT=zeros_t[:, :128], rhs=zeros_t[:],
                    start=True, stop=False, skip_group_check=True,
                )
                mm = ent["mm"]
                nmm = len(mm)
                for g0 in range(0, nmm, SGROUP):
                    gn = min(SGROUP, nmm - g0)
                    col0 = mm[g0][2]
                    s_t = s_pool.tile([128, SGROUP, TILE], dt.float16,
                                      tag="smat")
                    nc.vector.tensor_tensor(
                        out=s_t[:, :gn, :],
                        in0=_bc_mid(iota_t[:], gn),
                        in1=dstloc_t[:, col0 : col0 + gn].to_broadcast(
                            [128, gn, TILE]
                        ),
                        op=mybir.AluOpType.is_equal,
                    )
                    gi = 0
                    while gi < gn:
                        buf_ci, j, _, is_self = mm[g0 + gi]
                        run = 1
                        while (
                            gi + run < gn
                            and mm[g0 + gi + run][0] == buf_ci
                            and mm[g0 + gi + run][1] == j + run
                        ):
                            run += 1
                        lhsT = (
                            xself_t[:, b * 4 + buf_ci, :]
                            if is_self
                            else buf[:, buf_ci, :]
                        )
                        # same lhsT over the run: one wide matmul, one
                        # LDWEIGHTS (s_t middle dim is contiguous)
                        s_base = s_t[:, gi, :]
                        rhs = bass.AP(
                            s_base.tensor,
                            s_base.offset,
                            [s_base.ap[0], [1, run * TILE]],
                        )
                        nc.tensor.matmul(
                            out=agg_ps[:, j * TILE : (j + run) * TILE],
                            lhsT=lhsT,
                            rhs=rhs,
                            start=False,
                            stop=(g0 + gi + run == nmm),
                            skip_group_check=True,
                        )
                        gi += run

                # epilogue runs one batch behind so the in-order PE stream
                # never stalls on the cross-engine copy/relu roundtrips
                pend.append((b, bw, agg_ps))
                if len(pend) > 1:
                    _epilogue(*pend.pop(0))
            _epilogue(*pend.pop(0))

    nc.compile()
    return nc


_CACHE = {}
last_results = None


def kernel(x, edge_index, W1, b1, W2, b2):
    import os

    meta, xs, xself, dstloc, srcidx, dinvB = _preprocess(x, edge_index)

    iota = np.broadcast_to(np.arange(128, dtype=F16), (128, 128)).copy()
    w1t = np.asarray(W1, dtype=F32).T.copy()              # [D, HID]
    b1c = np.asarray(b1, dtype=F32).reshape(HID, 1)
    w2t = np.asarray(W2, dtype=F32).T.astype(F16).copy()  # [HID, ODIM]
    b2c = np.asarray(b2, dtype=F32).reshape(ODIM, 1)

    key = tuple(
        (e["bw"], tuple(e["mm"]))
        + tuple(sd["k"] for sd in e["sides"])
        for e in meta["batches"]
    )
    if key not in _CACHE:
        _CACHE[key] = _build_program(meta)
    nc = _CACHE[key]

    in_maps = []
    for c in range(CORES):
        in_maps.append(
            {
                "xs": xs,
                "xself": xself[c],
                "dstloc": dstloc[c],
                "srcidx": srcidx[c],
                "dinvB": dinvB[c],
                "iota": iota,
                "w1t": w1t,
                "b1c": b1c,
                "w2t": w2t,
                "b2c": b2c,
            }
        )

    trace = bool(os.environ.get("GCN_TRACE"))
    res = run_bass_kernel_spmd(
        nc, in_maps, core_ids=list(range(CORES)), trace=trace
    )
    global last_results
    last_results = res
    big = np.concatenate([res.results[c]["out"] for c in range(CORES)], axis=1)
    return np.ascontiguousarray(big[:, :N].T).astype(F32)
